# revision 4
# baseline (speedup 1.0000x reference)
"""GNN mean-aggregation conv kernel for Trainium2, 8-core SPMD.

Computes out[v] = (1/deg[v]) * sum_{(s,v) in E} (x[s] @ W.T + b), deg by dst.

Strategy: shard destination nodes across 8 cores (12500 rows each).  The host
pre-packs, per core, the edge source features in "staircase slot" layout:
dsts are sorted by in-degree (ascending) into chunks of 512, and re-sorted
descending inside each chunk, so slot s of chunk k holds the n[k][s] dsts
with deg > s as a contiguous run of feature columns x[src]^T * inv_deg[dst]
(bf16, feature-major; run widths shared across cores as the max).  The device
streams the packed array sequentially — no gather, ~1% padding.  Chunks are
batched into super-group DMAs of ramping size.  Per chunk the PE accumulates
the slot runs with stationary W^T into one PSUM bank (h^T[j, d] staircase),
chunk 0 (the only one with deg-0 dsts) starts with a rank-1 b*mask matmul,
all other chunks get + b as a per-partition bias in the DVE copy that drains
PSUM to bf16.  The host transposes and un-permutes the result.
"""

import numpy as np
import ml_dtypes

BF16 = ml_dtypes.bfloat16

N, E, D = 100000, 640000, 128
NCORES = 8
NPC = N // NCORES            # dst nodes per core
P = 128                      # partition dim
CW = 512                     # dsts per chunk (one PSUM bank of fp32)
NCH = (NPC + CW - 1) // CW   # 25 chunks per core
NPADC = NCH * CW             # 12800 padded dst rows per core
GROUP_COLS_START = 2048      # first super-group size (cols), doubles to cap
GROUP_COLS_CAP = 16384


def _build_schedule(edge_index):
    """Host-side prep: per-core degree-sorted staircase packing.

    Returns (sched, per_core): sched has the shared per-chunk slot widths
    `nbar` and DMA super-grouping; per_core holds {ids, scale, bmask, order}.
    """
    src = np.asarray(edge_index[0], dtype=np.int64)
    dst = np.asarray(edge_index[1], dtype=np.int64)

    deg = np.bincount(dst, minlength=N)
    inv_deg = np.where(deg > 0, 1.0 / np.maximum(deg, 1), 0.0).astype(np.float32)

    # slot index of each edge within its dst (stable order)
    ord_e = np.argsort(dst, kind="stable")
    ks = dst[ord_e]
    first = np.concatenate([[0], np.nonzero(np.diff(ks))[0] + 1])
    run_id = np.zeros(E, dtype=np.int64)
    run_id[first[1:]] = 1
    run_id = np.cumsum(run_id)
    slot_sorted = np.arange(E) - first[run_id]
    slot = np.empty(E, dtype=np.int64)
    slot[ord_e] = slot_sorted

    core = dst // NPC
    dstl = dst - core * NPC

    per_core = []
    n_all = []
    for c in range(NCORES):
        degp = np.zeros(NPADC, dtype=np.int64)
        degp[:NPC] = deg[c * NPC : (c + 1) * NPC]
        o1 = np.argsort(degp, kind="stable")           # ascending across chunks
        order = np.empty(NPADC, dtype=np.int64)
        ncore = []
        for k in range(NCH):
            mem = o1[k * CW : (k + 1) * CW]
            od = mem[np.argsort(-degp[mem], kind="stable")]  # desc within chunk
            order[k * CW : (k + 1) * CW] = od
            dsc = degp[od]
            S = max(int(dsc[0]), 1)
            hist = np.bincount(dsc, minlength=S + 1)
            tail = np.cumsum(hist[::-1])[::-1]         # tail[s] = #{deg >= s}
            ncore.append([int(tail[s + 1]) for s in range(S)])
        n_all.append(ncore)
        pos = np.empty(NPADC, dtype=np.int64)
        pos[order] = np.arange(NPADC)
        per_core.append({"order": order, "pos": pos, "degp_sorted": degp[order]})

    # shared slot widths: max over cores (lists may differ in length)
    nbar = []
    for k in range(NCH):
        S = max(len(n_all[c][k]) for c in range(NCORES))
        row = []
        for s in range(S):
            row.append(max(n_all[c][k][s] if s < len(n_all[c][k]) else 0
                           for c in range(NCORES)))
        row = [max(w, 1) for w in row]
        nbar.append(row)

    # column base per (chunk, slot)
    cb = []
    t = 0
    for k in range(NCH):
        row = []
        for w in nbar[k]:
            row.append(t)
            t += w
        cb.append(row)
    TOT = t

    # DMA super-groups: consecutive chunks, ramping size cap
    groups = []
    cap = GROUP_COLS_START
    k = 0
    while k < NCH:
        g = []
        cols = 0
        while k < NCH and (not g or cols + sum(nbar[k]) <= cap):
            g.append(k)
            cols += sum(nbar[k])
            k += 1
        groups.append(g)
        cap = min(cap * 2, GROUP_COLS_CAP)

    for c in range(NCORES):
        pc = per_core[c]
        m = core == c
        p_e = pc["pos"][dstl[m]]
        k_e = p_e // CW
        r_e = p_e % CW                       # rank within chunk (desc by deg)
        s_e = slot[m]
        cb_flat = np.array(
            [cb[k][s] if s < len(cb[k]) else 0
             for k in range(NCH) for s in range(len(cb[k]))]
        )
        # flat (k, s) -> cb lookup
        srow = np.concatenate([[0], np.cumsum([len(r) for r in cb])])
        col = cb_flat[srow[k_e] + s_e] + r_e
        ids = np.zeros(TOT, dtype=np.int64)
        scale = np.zeros(TOT, dtype=np.float32)
        ids[col] = src[m]
        scale[col] = inv_deg[dst[m]]
        bmask = (pc["degp_sorted"][:CW] > 0).astype(np.float32).reshape(1, CW)
        pc["ids"] = ids
        pc["scale"] = scale
        pc["bmask"] = bmask

    return {"nbar": nbar, "cb": cb, "TOT": TOT, "groups": groups}, per_core


def _build_program(sched):
    import concourse.tile as tile
    from concourse import bacc, mybir

    f32 = mybir.dt.float32
    bf16 = mybir.dt.bfloat16

    nbar = sched["nbar"]
    TOT = sched["TOT"]
    groups = sched["groups"]

    nc = bacc.Bacc(
        "TRN2",
        target_bir_lowering=False,
        debug=False,
        enable_asserts=False,
        num_devices=NCORES,
    )

    pt_d = nc.dram_tensor("pt", [P, TOT], bf16, kind="ExternalInput").ap()
    bmask_d = nc.dram_tensor("bmask", [1, CW], bf16, kind="ExternalInput").ap()
    wt_d = nc.dram_tensor("wt", [D, D], bf16, kind="ExternalInput").ap()
    brow_d = nc.dram_tensor("brow", [1, D], bf16, kind="ExternalInput").ap()
    bcol_d = nc.dram_tensor("bcol", [P, 1], f32, kind="ExternalInput").ap()
    outt_d = nc.dram_tensor("outt", [P, NPADC], bf16, kind="ExternalOutput").ap()

    with tile.TileContext(nc) as tc:
        with (
            tc.tile_pool(name="const", bufs=1) as cpool,
            tc.tile_pool(name="g", bufs=3) as gpool,
            tc.tile_pool(name="stage", bufs=4) as stpool,
            tc.tile_pool(name="ph", bufs=4, space="PSUM") as ppool,
        ):
            wt_s = cpool.tile([D, D], bf16)
            nc.sync.dma_start(wt_s[:], wt_d[:, :])
            brow_s = cpool.tile([1, D], bf16)
            nc.sync.dma_start(brow_s[:], brow_d[:, :])
            bmask_s = cpool.tile([1, CW], bf16)
            nc.sync.dma_start(bmask_s[:], bmask_d[:, :])
            bcol_s = cpool.tile([P, 1], f32)
            nc.sync.dma_start(bcol_s[:], bcol_d[:, :])

            base = 0
            for grp in groups:
                gcols = sum(sum(nbar[k]) for k in grp)
                g = gpool.tile([P, gcols], bf16, tag="g")
                nc.sync.dma_start(g[:], pt_d[:, base : base + gcols])
                off = 0
                for k in grp:
                    widths = nbar[k]
                    ph = ppool.tile([P, CW], f32, tag="ph")
                    if k == 0:
                        # deg-0 dsts live here: b enters via rank-1 b*mask
                        nc.tensor.matmul(
                            out=ph[:],
                            lhsT=brow_s[:],
                            rhs=bmask_s[:, :],
                            start=True,
                            stop=False,
                        )
                    for s, w in enumerate(widths):
                        nc.tensor.matmul(
                            out=ph[:, :w],
                            lhsT=wt_s[:],
                            rhs=g[:, off : off + w],
                            start=(s == 0 and k != 0),
                            stop=(s == len(widths) - 1),
                        )
                        off += w
                    st = stpool.tile([P, CW], bf16, tag="st")
                    if k == 0:
                        nc.scalar.copy(st[:], ph[:])
                    else:
                        nc.vector.tensor_scalar(
                            out=st[:],
                            in0=ph[:],
                            scalar1=bcol_s[:, 0:1],
                            scalar2=None,
                            op0=mybir.AluOpType.add,
                        )
                    nc.scalar.dma_start(outt_d[:, k * CW : (k + 1) * CW], st[:])
                base += gcols

    nc.compile()
    return nc


_CACHED = None


def _get_program(sched):
    global _CACHED
    key = repr((sched["nbar"], sched["groups"])).encode()
    if _CACHED is not None and _CACHED[0] == key:
        return _CACHED[1]
    nc = _build_program(sched)
    _CACHED = (key, nc)
    return nc


def _pack_inputs(x, W, b, per_core):
    """Build per-core device input arrays from the schedule."""
    xT = np.ascontiguousarray(np.asarray(x, dtype=np.float32).T)  # [D, N]
    wt = np.ascontiguousarray(np.asarray(W, dtype=np.float32).T).astype(BF16)
    brow = np.asarray(b, dtype=np.float32).reshape(1, D).astype(BF16)
    bcol = np.asarray(b, dtype=np.float32).reshape(P, 1)
    in_maps = []
    for pc in per_core:
        pt = (xT[:, pc["ids"]] * pc["scale"][None, :]).astype(BF16)
        in_maps.append(
            {
                "pt": np.ascontiguousarray(pt),
                "bmask": pc["bmask"].astype(BF16),
                "wt": wt,
                "brow": brow,
                "bcol": bcol,
            }
        )
    return in_maps


LAST_RESULTS = None


def kernel(x, edge_index, W, b, _trace=False):
    global LAST_RESULTS
    from concourse.bass_utils import run_bass_kernel_spmd

    sched, per_core = _build_schedule(edge_index)
    nc = _get_program(sched)
    in_maps = _pack_inputs(x, W, b, per_core)

    res = run_bass_kernel_spmd(
        nc, in_maps, core_ids=list(range(NCORES)), trace=_trace
    )
    LAST_RESULTS = res
    out = np.empty((N, D), dtype=np.float32)
    for c in range(NCORES):
        outc = np.asarray(res.results[c]["outt"]).astype(np.float32).T  # [NPADC, D]
        order = per_core[c]["order"]
        valid = order < NPC
        out[c * NPC + order[valid]] = outc[valid]
    return out


# revision 6
# speedup vs baseline: 1.2156x; 1.2156x over previous
"""GNN mean-aggregation conv kernel for Trainium2, 8-core SPMD.

Computes out[v] = (1/deg[v]) * sum_{(s,v) in E} (x[s] @ W.T + b), deg by dst.

Strategy: shard destination nodes across 8 cores (12500 rows each).  The host
pre-packs, per core, the edge source features in "staircase slot" layout:
dsts are sorted by in-degree (ascending) into chunks of 512, and re-sorted
descending inside each chunk, so slot s of chunk k holds the n[k][s] dsts
with deg > s as a contiguous run of feature columns x[src]^T * inv_deg[dst]
(bf16, feature-major; run widths shared across cores as the max).  The device
streams the packed array sequentially — no gather, ~1% padding.  Chunks are
batched into super-group DMAs of ramping size.  Per chunk the PE accumulates
the slot runs with stationary W^T into one PSUM bank (h^T[j, d] staircase),
chunk 0 (the only one with deg-0 dsts) starts with a rank-1 b*mask matmul,
all other chunks get + b as a per-partition bias in the DVE copy that drains
PSUM to bf16.  The host transposes and un-permutes the result.
"""

import numpy as np
import ml_dtypes

BF16 = ml_dtypes.bfloat16

N, E, D = 100000, 640000, 128
NCORES = 8
NPC = N // NCORES            # dst nodes per core
P = 128                      # partition dim
CW = 512                     # dsts per chunk (one PSUM bank of fp32)
NCH = (NPC + CW - 1) // CW   # 25 chunks per core
NPADC = NCH * CW             # 12800 padded dst rows per core
GROUP_COLS_START = 2048      # first super-group size (cols), doubles to cap
GROUP_COLS_CAP = 16384


def _build_schedule(edge_index):
    """Host-side prep: per-core degree-sorted staircase packing.

    Returns (sched, per_core): sched has the shared per-chunk slot widths
    `nbar` and DMA super-grouping; per_core holds {ids, scale, bmask, order}.
    """
    src = np.asarray(edge_index[0], dtype=np.int64)
    dst = np.asarray(edge_index[1], dtype=np.int64)

    deg = np.bincount(dst, minlength=N)
    inv_deg = np.where(deg > 0, 1.0 / np.maximum(deg, 1), 0.0).astype(np.float32)

    # slot index of each edge within its dst (stable order)
    ord_e = np.argsort(dst, kind="stable")
    ks = dst[ord_e]
    first = np.concatenate([[0], np.nonzero(np.diff(ks))[0] + 1])
    run_id = np.zeros(E, dtype=np.int64)
    run_id[first[1:]] = 1
    run_id = np.cumsum(run_id)
    slot_sorted = np.arange(E) - first[run_id]
    slot = np.empty(E, dtype=np.int64)
    slot[ord_e] = slot_sorted

    core = dst // NPC
    dstl = dst - core * NPC

    per_core = []
    n_all = []
    for c in range(NCORES):
        degp = np.zeros(NPADC, dtype=np.int64)
        degp[:NPC] = deg[c * NPC : (c + 1) * NPC]
        o1 = np.argsort(degp, kind="stable")           # ascending across chunks
        order = np.empty(NPADC, dtype=np.int64)
        ncore = []
        for k in range(NCH):
            mem = o1[k * CW : (k + 1) * CW]
            od = mem[np.argsort(-degp[mem], kind="stable")]  # desc within chunk
            order[k * CW : (k + 1) * CW] = od
            dsc = degp[od]
            S = max(int(dsc[0]), 1)
            hist = np.bincount(dsc, minlength=S + 1)
            tail = np.cumsum(hist[::-1])[::-1]         # tail[s] = #{deg >= s}
            ncore.append([int(tail[s + 1]) for s in range(S)])
        n_all.append(ncore)
        pos = np.empty(NPADC, dtype=np.int64)
        pos[order] = np.arange(NPADC)
        per_core.append({"order": order, "pos": pos, "degp_sorted": degp[order]})

    # shared slot widths: max over cores (lists may differ in length)
    nbar = []
    for k in range(NCH):
        S = max(len(n_all[c][k]) for c in range(NCORES))
        row = []
        for s in range(S):
            row.append(max(n_all[c][k][s] if s < len(n_all[c][k]) else 0
                           for c in range(NCORES)))
        row = [max(w, 1) for w in row]
        nbar.append(row)

    # column base per (chunk, slot)
    cb = []
    t = 0
    for k in range(NCH):
        row = []
        for w in nbar[k]:
            row.append(t)
            t += w
        cb.append(row)
    TOT = t

    # one DMA per chunk: the ascending-degree order gives a natural
    # small-to-large ramp, and fine granularity keeps PE and DMA overlapped
    groups = [[k] for k in range(NCH)]

    for c in range(NCORES):
        pc = per_core[c]
        m = core == c
        p_e = pc["pos"][dstl[m]]
        k_e = p_e // CW
        r_e = p_e % CW                       # rank within chunk (desc by deg)
        s_e = slot[m]
        cb_flat = np.array(
            [cb[k][s] if s < len(cb[k]) else 0
             for k in range(NCH) for s in range(len(cb[k]))]
        )
        # flat (k, s) -> cb lookup
        srow = np.concatenate([[0], np.cumsum([len(r) for r in cb])])
        col = cb_flat[srow[k_e] + s_e] + r_e
        ids = np.zeros(TOT, dtype=np.int64)
        scale = np.zeros(TOT, dtype=np.float32)
        ids[col] = src[m]
        scale[col] = inv_deg[dst[m]]
        bmask = (pc["degp_sorted"][:CW] > 0).astype(np.float32).reshape(1, CW)
        pc["ids"] = ids
        pc["scale"] = scale
        pc["bmask"] = bmask

    return {"nbar": nbar, "cb": cb, "TOT": TOT, "groups": groups}, per_core


def _build_program(sched):
    import concourse.tile as tile
    from concourse import bacc, mybir

    f32 = mybir.dt.float32
    bf16 = mybir.dt.bfloat16

    nbar = sched["nbar"]
    TOT = sched["TOT"]
    groups = sched["groups"]

    nc = bacc.Bacc(
        "TRN2",
        target_bir_lowering=False,
        debug=False,
        enable_asserts=False,
        num_devices=NCORES,
    )

    pt_d = nc.dram_tensor("pt", [P, TOT], bf16, kind="ExternalInput").ap()
    bmask_d = nc.dram_tensor("bmask", [1, CW], bf16, kind="ExternalInput").ap()
    wt_d = nc.dram_tensor("wt", [D, D], bf16, kind="ExternalInput").ap()
    brow_d = nc.dram_tensor("brow", [1, D], bf16, kind="ExternalInput").ap()
    bcol_d = nc.dram_tensor("bcol", [P, 1], f32, kind="ExternalInput").ap()
    outt_d = nc.dram_tensor("outt", [P, NPADC], bf16, kind="ExternalOutput").ap()

    with tile.TileContext(nc) as tc:
        with (
            tc.tile_pool(name="const", bufs=1) as cpool,
            tc.tile_pool(name="g", bufs=6) as gpool,
            tc.tile_pool(name="stage", bufs=4) as stpool,
            tc.tile_pool(name="ph", bufs=4, space="PSUM") as ppool,
        ):
            wt_s = cpool.tile([D, D], bf16)
            nc.sync.dma_start(wt_s[:], wt_d[:, :])
            brow_s = cpool.tile([1, D], bf16)
            nc.sync.dma_start(brow_s[:], brow_d[:, :])
            bmask_s = cpool.tile([1, CW], bf16)
            nc.sync.dma_start(bmask_s[:], bmask_d[:, :])
            bcol_s = cpool.tile([P, 1], f32)
            nc.sync.dma_start(bcol_s[:], bcol_d[:, :])

            base = 0
            for grp in groups:
                gcols = sum(sum(nbar[k]) for k in grp)
                g = gpool.tile([P, gcols], bf16, tag="g")
                nc.sync.dma_start(g[:], pt_d[:, base : base + gcols])
                off = 0
                for k in grp:
                    widths = nbar[k]
                    ph = ppool.tile([P, CW], f32, tag="ph")
                    if k == 0:
                        # deg-0 dsts live here: b enters via rank-1 b*mask
                        nc.tensor.matmul(
                            out=ph[:],
                            lhsT=brow_s[:],
                            rhs=bmask_s[:, :],
                            start=True,
                            stop=False,
                        )
                    for s, w in enumerate(widths):
                        nc.tensor.matmul(
                            out=ph[:, :w],
                            lhsT=wt_s[:],
                            rhs=g[:, off : off + w],
                            start=(s == 0 and k != 0),
                            stop=(s == len(widths) - 1),
                        )
                        off += w
                    st = stpool.tile([P, CW], bf16, tag="st")
                    if k == 0:
                        nc.scalar.copy(st[:], ph[:])
                    else:
                        nc.vector.tensor_scalar(
                            out=st[:],
                            in0=ph[:],
                            scalar1=bcol_s[:, 0:1],
                            scalar2=None,
                            op0=mybir.AluOpType.add,
                        )
                    nc.scalar.dma_start(outt_d[:, k * CW : (k + 1) * CW], st[:])
                base += gcols

    nc.compile()
    return nc


_CACHED = None


def _get_program(sched):
    global _CACHED
    key = repr((sched["nbar"], sched["groups"])).encode()
    if _CACHED is not None and _CACHED[0] == key:
        return _CACHED[1]
    nc = _build_program(sched)
    _CACHED = (key, nc)
    return nc


def _pack_inputs(x, W, b, per_core):
    """Build per-core device input arrays from the schedule."""
    xT = np.ascontiguousarray(np.asarray(x, dtype=np.float32).T)  # [D, N]
    wt = np.ascontiguousarray(np.asarray(W, dtype=np.float32).T).astype(BF16)
    brow = np.asarray(b, dtype=np.float32).reshape(1, D).astype(BF16)
    bcol = np.asarray(b, dtype=np.float32).reshape(P, 1)
    in_maps = []
    for pc in per_core:
        pt = (xT[:, pc["ids"]] * pc["scale"][None, :]).astype(BF16)
        in_maps.append(
            {
                "pt": np.ascontiguousarray(pt),
                "bmask": pc["bmask"].astype(BF16),
                "wt": wt,
                "brow": brow,
                "bcol": bcol,
            }
        )
    return in_maps


LAST_RESULTS = None


def kernel(x, edge_index, W, b, _trace=False):
    global LAST_RESULTS
    from concourse.bass_utils import run_bass_kernel_spmd

    sched, per_core = _build_schedule(edge_index)
    nc = _get_program(sched)
    in_maps = _pack_inputs(x, W, b, per_core)

    res = run_bass_kernel_spmd(
        nc, in_maps, core_ids=list(range(NCORES)), trace=_trace
    )
    LAST_RESULTS = res
    out = np.empty((N, D), dtype=np.float32)
    for c in range(NCORES):
        outc = np.asarray(res.results[c]["outt"]).astype(np.float32).T  # [NPADC, D]
        order = per_core[c]["order"]
        valid = order < NPC
        out[c * NPC + order[valid]] = outc[valid]
    return out
